# revision 29
# baseline (speedup 1.0000x reference)
"""Longformer sliding-window self-attention on 8 Trainium2 NeuronCores.

Problem: hidden_states [2, 4096, 1024], 16 heads x 64 dim, window w=256.
  q = (X@Wq + bq)/sqrt(64); k = X@Wk + bk; v = X@Wv + bv
  Banded attention: query p attends keys in [p-256, p+256] (inclusive),
  clipped to the sequence.

Sharding: 2 batches x 4 head-groups. Core r handles batch r//4 and heads
(r%4)*4..+4 (256 projection columns, stored as 2 sub-slices of 128).
Host concatenates the per-core [4096, 256] outputs.

Device-side design (per core):
  P1: X^T chunks [128, 8kc, 512] stream in; Q^T/K^T (bf16) land in
      transposed stores via f32r matmuls (N=512, 1 PE cycle/row); V^T is
      PE-transposed (bf16, 1 cycle/row) into natural [key, dim] chunks
      with a ones-column so the PV matmul also emits the softmax
      denominator Z.
  P2: per 128-query tile, 5 key chunks of 128 cover the band exactly.
      Scores S^T [key, query] via bf16 matmuls (bf16 runs 1 cycle/row at
      any moving size; f32r would be 4x slower below N=256).  Band masks
      are added by the idle Pool engine directly in PSUM; one Act exp
      yields bf16 probs; PV uses the probs as the STATIONARY operand so
      attn lands in natural [query, dim] layout with 65 moving columns
      and no transposes.  DVE divides by the Z column (stride-0
      broadcast) and batches 4 tiles per output DMA.
  P1 and P2 are interleaved (a 3-slot PSUM software pipeline, scores and
  PV emitted 2 steps apart) so Act/Pool/DVE latency hides behind PE work.
"""

import os
import numpy as np
from collections import deque

import concourse.bass as bass
import concourse.mybir as mybir
import concourse.tile as tile
from concourse.vector_clock import ScopedClock
from concourse.bass_utils import run_bass_kernel_spmd
from contextlib import ExitStack
from ml_dtypes import bfloat16

# Problem shape (hardcoded per the harness contract).
B, S, E = 2, 4096, 1024
H, D, W = 16, 64, 256
NCORE = 8
HPC = 4              # heads per core
C = HPC * D          # 256 projection output columns per core
NSUB = C // 128      # 2 store sub-slices
TC = 512             # projection token-chunk
NT = S // TC         # 8 projection chunks (single batch per core)
KCH = E // 128       # 8 contraction chunks of the projection
SP = S + 2 * W       # padded key extent (offset +W)
NCH = SP // 128      # 36 padded key chunks; valid j in [2, 34)
NTILE = S // 128     # 32 query tiles of 128
MASKVAL = -1e9
SCALE = 1.0 / np.sqrt(D)

f32 = mybir.dt.float32
f32r = mybir.dt.float32r
bf16 = mybir.dt.bfloat16
AF = mybir.ActivationFunctionType
Alu = mybir.AluOpType


class _TileContext(tile.TileContext):
    """TileContext whose exit drain splits semaphore waits.

    The walrus build in this container rejects >1 sync wait on one
    instruction ("Too many sync wait commands"), while Tile's exit drain
    accumulates one wait per outstanding semaphore.  Carry each wait on its
    own drain instruction instead.
    """

    MAX_WAITS = 1

    def _drain_and_barrier(self, tick_clock, wait_clock):
        drain_inst = self.nc.sync.drain()
        wait_clock.add_sem_waits(
            drain_inst.ins, ScopedClock({None: tick_clock.global_clock})
        )
        si = drain_inst.ins.sync_info
        waits = list(si.on_wait or []) if si is not None else []
        if len(waits) > self.MAX_WAITS:
            si.on_wait = waits[: self.MAX_WAITS]
            rest = waits[self.MAX_WAITS :]
            while rest:
                d2 = self.nc.sync.drain()
                si2 = d2.ins.sync_info
                if si2 is None:
                    si2 = mybir.SyncInfo(on_wait=[], on_update=[])
                    d2.ins.sync_info = si2
                si2.on_wait = rest[: self.MAX_WAITS]
                rest = rest[self.MAX_WAITS :]
        self.nc.all_engine_barrier()
        assert self.sems is not None
        popped = self.nc._tile_sem_poison_stack.pop()
        assert popped is self._sem_poison
        self.nc.clear_and_free_semaphores(list(self.sems.allocated().values()))
        self.nc.all_engine_barrier()


def _split_sync_waits(nc, limit=1):
    """Move excess per-instruction sem waits onto same-engine NoOp carriers."""
    n_new = 0
    for fn in nc.m.functions:
        for bb in fn.blocks:
            out = []
            for inst in bb.instructions:
                si = getattr(inst, "sync_info", None)
                waits = list(si.on_wait) if si is not None and si.on_wait else []
                if len(waits) > limit:
                    extra = waits[: len(waits) - limit]
                    si.on_wait = waits[len(waits) - limit :]
                    while extra:
                        chunk = extra[:limit]
                        extra = extra[limit:]
                        nop = mybir.InstNoOp(
                            name=f"waitsplit-{nc.next_id()}", ins=[], outs=[]
                        )
                        nop.engine = inst.engine
                        nop.sync_info = mybir.SyncInfo(on_wait=chunk, on_update=[])
                        out.append(nop)
                        n_new += 1
                out.append(inst)
            bb.instructions = out
    return n_new


def _declare_aps(nc):
    return {
        "xt": nc.dram_tensor("xt", [E, S], bf16, kind="ExternalInput").ap(),
        "wq": nc.dram_tensor("wq", [E, C], bf16, kind="ExternalInput").ap(),
        "bq": nc.dram_tensor("bq", [C], f32, kind="ExternalInput").ap(),
        "wk": nc.dram_tensor("wk", [E, C], bf16, kind="ExternalInput").ap(),
        "bk": nc.dram_tensor("bk", [C], f32, kind="ExternalInput").ap(),
        "wv": nc.dram_tensor("wv", [E, C], bf16, kind="ExternalInput").ap(),
        "bv": nc.dram_tensor("bv", [C], f32, kind="ExternalInput").ap(),
        "msk": nc.dram_tensor("msk", [2, 128, 128], bf16, kind="ExternalInput").ap(),
        "mskc": nc.dram_tensor("mskc", [128, 256], bf16, kind="ExternalInput").ap(),
        "idb": nc.dram_tensor("idb", [128, 128], bf16, kind="ExternalInput").ap(),
        "idn": nc.dram_tensor("idn", [128, 128], f32, kind="ExternalInput").ap(),
        "ones": nc.dram_tensor("ones", [1], bf16, kind="ExternalInput").ap(),
        "out": nc.dram_tensor("out", [S, C], f32, kind="ExternalOutput").ap(),
    }


def _make_pools(tc, ctx):
    return {
        "sing": ctx.enter_context(tc.tile_pool(name="sing", bufs=1)),
        "stores": ctx.enter_context(tc.tile_pool(name="stores", bufs=1)),
        "xp": ctx.enter_context(tc.tile_pool(name="xp", bufs=2)),
        "vtp": ctx.enter_context(tc.tile_pool(name="vtp", bufs=2)),
        "exp": ctx.enter_context(tc.tile_pool(name="exp", bufs=3)),
        "obp": ctx.enter_context(tc.tile_pool(name="obp", bufs=8)),
        # PSUM: psP = proj ring (2x2KB) + pvt (1x2KB) = 3 banks;
        #       psS = 3 slots x 3KB (scores strip + opsum) = 4.5 banks.
        "psP": ctx.enter_context(tc.tile_pool(name="psP", bufs=2, space="PSUM")),
        "psS": ctx.enter_context(tc.tile_pool(name="psS", bufs=2, space="PSUM")),
    }


def _setup(nc, tc, aps, P):
    """Constants + persistent stores.  Weight DMAs ride the Act queue and
    small constants the Pool queue so SP starts streaming X immediately."""
    sing = P["sing"]
    stores = P["stores"]
    cst = {}
    cst["idb"] = sing.tile([128, 128], bf16, name="idb")
    nc.scalar.dma_start(cst["idb"], aps["idb"])
    cst["idn"] = sing.tile([128, 128], f32, name="idn")
    nc.scalar.dma_start(cst["idn"], aps["idn"])
    cst["msk"] = sing.tile([128, 2, 128], bf16, name="msk")
    nc.scalar.dma_start(cst["msk"], aps["msk"].rearrange("m p x -> p m x"))
    cst["mskc"] = sing.tile([128, 256], bf16, name="mskc")
    nc.scalar.dma_start(cst["mskc"], aps["mskc"])

    cst["w"] = []
    cst["b"] = []
    for nm in ("q", "k", "v"):
        w_sb = sing.tile([128, KCH, C], bf16, name=f"w{nm}_sb")
        nc.scalar.dma_start(
            w_sb, aps["w" + nm].rearrange("(kc p) c -> p kc c", p=128)
        )
        b_sb = sing.tile([128, NSUB], f32, name=f"b{nm}_sb")
        nc.scalar.dma_start(
            b_sb, aps["b" + nm].rearrange("(s p) -> p s", p=128)
        )
        cst["w"].append(w_sb)
        cst["b"].append(b_sb)

    cst["QT"] = stores.tile([128, NSUB, S], bf16, name="QT")
    cst["KT"] = stores.tile([128, NSUB, SP], bf16, name="KT")
    cst["VS"] = stores.tile([128, HPC, NCH, D + 1], bf16, name="VS")
    # ones-column: softmax denominator accumulates through the PV matmul.
    ones_bcast = bass.AP(
        tensor=aps["ones"].tensor, offset=0, ap=[[0, 128], [0, NCH - 4]]
    )
    for h in range(HPC):
        nc.scalar.dma_start(cst["VS"][:, h, 2 : NCH - 2, D], ones_bcast)
    return cst


def _emit(nc, tc, aps, P, cst):
    STAGE = int(os.environ.get("KSTAGES", "4"))
    QT, KT, VS = cst["QT"], cst["KT"], cst["VS"]
    idb, idn, msk = cst["idb"], cst["idn"], cst["msk"]
    mskc = cst["mskc"]
    out_ap = aps["out"]
    xt_re = aps["xt"].rearrange("(kc p) n -> p kc n", p=128)

    P1L = os.environ.get("KP1", "full")  # q|qk|qkv|tr|full
    def emit_p1_chunk(t0, tc):
        nq = tc // 128
        xt_t = P["xp"].tile([128, KCH, TC], bf16, tag="xt", name="xt")
        nsplit = int(os.environ.get("KXSPLIT", "2"))
        kper = KCH // nsplit
        for part in range(nsplit):
            nc.sync.dma_start(
                xt_t[:, kper * part : kper * (part + 1), 0:tc],
                xt_re[:, kper * part : kper * (part + 1), t0 : t0 + tc],
            )
        for s in range(NSUB):
            col = slice(s * 128, (s + 1) * 128)
            for ip, nm in enumerate("qkv"):
                ps = P["psP"].tile([128, TC], f32, tag="ps", name=f"ps{nm}")
                for kc in range(KCH):
                    nc.tensor.matmul(
                        ps[:, 0:tc],
                        cst["w"][ip][:, kc, col],
                        xt_t[:, kc, 0:tc],
                        start=(kc == 0),
                        stop=(kc == KCH - 1),
                    )
                if nm == "q":
                    nc.vector.tensor_scalar_add(
                        QT[:, s, t0 : t0 + tc],
                        ps[:, 0:tc],
                        cst["b"][0][:, s : s + 1],
                    )
                    if P1L == "q":
                        break
                elif nm == "k":
                    nc.scalar.activation(
                        KT[:, s, W + t0 : W + t0 + tc],
                        ps[:, 0:tc],
                        AF.Identity,
                        bias=cst["b"][1][:, s : s + 1],
                    )
                    if P1L == "qk":
                        break
                else:
                    vt = P["vtp"].tile([128, TC], bf16, tag="vt", name="vt")
                    nc.scalar.activation(
                        vt[:, 0:tc], ps[:, 0:tc], AF.Identity,
                        bias=cst["b"][2][:, s : s + 1],
                    )
                    if P1L == "qkv":
                        continue
                    ch0 = 2 + t0 // 128
                    # transpose both heads at once: out[tok, col] =
                    # sum_k vt[k, tok] * I[k, col] = vt[col, tok]
                    pvt = P["psP"].tile(
                        [128, 4, 128], f32, tag="ps", name="pvt"
                    )
                    for q4 in range(nq):
                        nc.tensor.matmul(
                            pvt[:, q4, :],
                            vt[:, q4 * 128 : (q4 + 1) * 128],
                            idb,
                            start=True,
                            stop=True,
                            skip_group_check=True,
                        )
                    if P1L == "tr":
                        continue
                    nc.vector.tensor_copy(
                        VS[:, 2 * s : 2 * s + 2, ch0 : ch0 + nq, 0:D],
                        pvt[:, 0:nq, :].rearrange("p c (h d) -> p h c d", h=2),
                    )

    # --- P2 pipeline (tile pairs) ---
    # A pair = tiles (h, i) and (h, i+1) in ONE 3-bank PSUM slot:
    #   cols [0:640)    strip A   (banks 0-1)
    #   cols [640:1280) strip B   (banks 1-2)
    #   cols [1280:1345) opsum A, [1408:1473) opsum B (bank 2)
    # Scores run chunk-major across the pair so 4 of 10 matmuls reuse the
    # loaded stationary (same KT chunk); the 4 mask matmuls share idb; exp
    # handles both strips in one op for interior pairs.
    obufs = {}
    OB = int(os.environ.get("KOBATCH", "8"))
    STB = 640
    OPA, OPB = 1280, 1408

    def drange(i):
        return max(0, 2 - i), min(5, 34 - i)

    def emit_scores_pair(h, i):
        sub, rows = h // 2, slice((h % 2) * D, (h % 2) * D + D)
        dA = drange(i)
        dB = drange(i + 1)
        sl = P["psS"].tile([128, 1536], f32, tag="sl", name="sl")
        seen = set()
        mm = []  # (j, member, d, col)
        for d in range(dA[0], dA[1]):
            mm.append((i + d, 0, d, 128 * d))
        for d in range(dB[0], dB[1]):
            mm.append((i + 1 + d, 1, d, STB + 128 * d))
        mm.sort(key=lambda r: (r[0], r[1]))
        for j, member, d, col in mm:
            lo, hi = (dA, dB)[member]
            masked = (d == 0 and lo == 0) or (d == 4 and hi == 5)
            bank = col // 512
            nc.tensor.matmul(
                sl[:, col : col + 128],
                KT[rows, sub, j * 128 : (j + 1) * 128],
                QT[rows, sub, (i + member) * 128 : (i + member + 1) * 128],
                start=bank not in seen,
                stop=not masked,
                skip_group_check=True,
            )
            seen.add(bank)
        combined = dA[1] == 5 and dB[0] == 0
        if combined:
            # A-hi [512:640] and B-lo [640:768] are adjacent: one matmul
            nc.tensor.matmul(
                sl[:, 512:768], idb, mskc, start=False, stop=True,
                skip_group_check=True,
            )
        for member, (lo, hi) in enumerate((dA, dB)):
            off = member * STB
            if lo == 0 and not (combined and member == 1):
                nc.tensor.matmul(
                    sl[:, off : off + 128], idb, msk[:, 0], start=False,
                    stop=True, skip_group_check=True,
                )
            if hi == 5 and not (combined and member == 0):
                nc.tensor.matmul(
                    sl[:, off + 512 : off + 640], idb, msk[:, 1], start=False,
                    stop=True, skip_group_check=True,
                )
        if STAGE < 2:
            return (h, i, sl, None, dA, dB)
        ex = P["exp"].tile([128, 1280], bf16, tag="ex", name="ex")
        if dA == (0, 5) and dB == (0, 5):
            nc.scalar.activation(ex, sl[:, 0:1280], AF.Exp, scale=SCALE)
        else:
            nc.scalar.activation(
                ex[:, dA[0] * 128 : dA[1] * 128],
                sl[:, dA[0] * 128 : dA[1] * 128],
                AF.Exp,
                scale=SCALE,
            )
            nc.scalar.activation(
                ex[:, STB + dB[0] * 128 : STB + dB[1] * 128],
                sl[:, STB + dB[0] * 128 : STB + dB[1] * 128],
                AF.Exp,
                scale=SCALE,
            )
        return (h, i, sl, ex, dA, dB)

    def emit_pv_pair(st):
        h, i, sl, ex, dA, dB = st
        if STAGE < 3:
            return
        for member, (lo, hi) in enumerate((dA, dB)):
            op = (OPA, OPB)[member]
            for d in range(lo, hi):
                nc.tensor.matmul(
                    sl[:, op : op + 65],
                    ex[:, member * STB + d * 128 : member * STB + (d + 1) * 128],
                    VS[:, h, i + member + d, :],
                    start=(d == lo),
                    stop=(d == hi - 1),
                    skip_group_check=True,
                )
            ii = i + member
            if ii % OB == 0:
                obufs[h] = P["obp"].tile([128, OB, D], f32, tag="ob", name="ob")
            rc = P["obp"].tile([128, 1], f32, tag="rc", name="rc")
            nc.vector.reciprocal(rc, sl[:, op + 64 : op + 65])
            nc.vector.tensor_scalar_mul(
                obufs[h][:, ii % OB, :], sl[:, op : op + 64], rc
            )
            if ii % OB == OB - 1 and STAGE >= 4:
                i0 = ii - (OB - 1)
                dst = bass.AP(
                    tensor=out_ap.tensor,
                    offset=(i0 * 128) * C + h * D,
                    ap=[[C, 128], [128 * C, OB], [1, D]],
                )
                nc.sync.dma_start(dst, obufs[h])

    # P1 token schedule: coarse 512-chunks, then 256-chunks at the end so
    # late query-tile pairs unlock before the final tokens land.
    sched = [(t * 512, 512) for t in range(NT)]
    # pair (i, i+1) ready once tokens < 128*(j_max+1) - 256 are projected
    def toks_needed(i):
        return 128 * (min(i + 5, 33) + 1) - 256

    pend = deque()
    done = 0
    for t0, tc in sched:
        emit_p1_chunk(t0, tc)
        prev, done = done, t0 + tc
        if STAGE < 1:
            continue
        for i in range(0, NTILE, 2):
            if prev < toks_needed(i) <= done:
                for h in range(HPC):
                    pend.append(emit_scores_pair(h, i))
                    if len(pend) > 1:
                        emit_pv_pair(pend.popleft())
    while pend:
        emit_pv_pair(pend.popleft())


def build_program(split_waits=False, loop_n=0):
    nc = bass.Bass("TRN2", target_bir_lowering=False, debug=False)
    aps = _declare_aps(nc)
    with _TileContext(nc) as tc, ExitStack() as ctx:
        P = _make_pools(tc, ctx)
        cst = _setup(nc, tc, aps, P)

        def body():
            _emit(nc, tc, aps, P, cst)

        if loop_n > 0:
            with tc.For_i(0, loop_n, 1):
                body()
        else:
            body()
    if split_waits:
        _split_sync_waits(nc)
    return nc


def _band_masks():
    """Additive triangle masks [2, 128, 128] for the d=0 / d=4 key chunks.

    d=0: key-query offset y'-x' in [-127,127], valid iff y' >= x'.
    d=4: offset-512 in [-127,127], valid iff y' <= x'.
    """
    yy = np.arange(128, dtype=np.int64)[:, None]
    xx = np.arange(128, dtype=np.int64)[None, :]
    m_lo = np.where(yy >= xx, 0.0, MASKVAL)
    m_hi = np.where(yy <= xx, 0.0, MASKVAL)
    return np.stack([m_lo, m_hi]).astype(np.float32).astype(bfloat16)


def make_in_maps(hidden_states, Wq, bq, Wk, bk, Wv, bv):
    hs = np.asarray(hidden_states, dtype=np.float32)
    xts = [np.ascontiguousarray(hs[b].T).astype(bfloat16) for b in range(B)]
    Wq = np.asarray(Wq, dtype=np.float32).astype(bfloat16)
    Wk = np.asarray(Wk, dtype=np.float32).astype(bfloat16)
    Wv = np.asarray(Wv, dtype=np.float32).astype(bfloat16)
    bq = np.asarray(bq, dtype=np.float32)
    bk = np.asarray(bk, dtype=np.float32)
    bv = np.asarray(bv, dtype=np.float32)
    msk = _band_masks()
    mskc = np.ascontiguousarray(
        np.concatenate([np.asarray(msk[1]), np.asarray(msk[0])], axis=1)
    )
    idb = np.eye(128, dtype=np.float32).astype(bfloat16)
    idn = np.eye(128, dtype=np.float32)
    ones = np.ones([1], dtype=np.float32).astype(bfloat16)
    in_maps = []
    for r in range(NCORE):
        b_r, g = divmod(r, NCORE // B)
        sl = slice(g * C, (g + 1) * C)
        in_maps.append(
            {
                "xt": xts[b_r],
                "wq": np.ascontiguousarray(Wq[:, sl]),
                "bq": np.ascontiguousarray(bq[sl]),
                "wk": np.ascontiguousarray(Wk[:, sl]),
                "bk": np.ascontiguousarray(bk[sl]),
                "wv": np.ascontiguousarray(Wv[:, sl]),
                "bv": np.ascontiguousarray(bv[sl]),
                "msk": msk,
                "mskc": mskc,
                "idb": idb,
                "idn": idn,
                "ones": ones,
            }
        )
    return in_maps


_NC_CACHE = {}


def kernel(hidden_states, Wq, bq, Wk, bk, Wv, bv):
    if "nc" not in _NC_CACHE:
        _NC_CACHE["nc"] = build_program(split_waits=True)
    nc = _NC_CACHE["nc"]
    in_maps = make_in_maps(hidden_states, Wq, bq, Wk, bk, Wv, bv)
    res = run_bass_kernel_spmd(nc, in_maps, core_ids=list(range(NCORE)))
    return assemble_out([res.results[r]["out"] for r in range(NCORE)])


def assemble_out(per_core):
    """8 x [S, C] -> [B, S, E]."""
    full = np.stack(
        [
            np.concatenate(per_core[b * (NCORE // B) : (b + 1) * (NCORE // B)], axis=1)
            for b in range(B)
        ]
    )
    return np.ascontiguousarray(full).astype(np.float32)
